# revision 31
# baseline (speedup 1.0000x reference)
"""Trainium2 Bass kernel for nn_Block_37383395345129 (sparse_attention).

Data-parallel over batch B=8: one batch element per NeuronCore. Params
replicated. Per core: LN1 -> QKV -> global+local window attention
(returns softmax probs as output #2) -> proj residual -> LN2 -> KAN MLP
(B-spline basis via shifted relu^3 identity folded into weights) ->
conv residual path -> combine.

Key device-side tricks:
  - S and S^T both computed on PE (bf16 operands, fp32 PSUM accumulate,
    1 cyc/col); softmax without max-subtraction (|S*scale| < 1.5); row
    sums r obtained free via a ones-column in the A@V matmul; attention
    probs written in one ACT pass as exp(scale*S - ln r) (bias =
    per-partition -ln r).
  - A@V computed transposed (X^T = V^T P^T) with 4 heads col-packed in
    one PSUM tile via tile_position; local window attention via a
    block-diagonal mask on the diagonal P^T blocks (gpsimd mul).
  - KAN spline: bases_k(x) = B3(2.5x+5.5-k'), B3 via 4th difference of
    relu^3 -> 12 shifted relu^3 channels, basis->weight fold done on
    host; cancellation-critical matmuls in full fp32.
  - Single ACT table set (exp/ln): silu = z*exp(-ln(1+exp(-z))),
    1/sigma = exp(-0.5*ln(var+eps)).
"""
import math
import ml_dtypes
import numpy as np

import concourse.bass as bass
import concourse.mybir as mybir
import concourse.tile as tile
from concourse import bacc
from concourse.bass_utils import run_bass_kernel_spmd

F32 = mybir.dt.float32
BF16 = mybir.dt.bfloat16
F32R = mybir.dt.float32r
AF = mybir.ActivationFunctionType
ALU = mybir.AluOpType

B, N, C = 8, 1024, 48
H, HD = 8, 6
SCALE = HD ** -0.5
WIN = 16
HID = 24
NS = 12          # number of relu^3 shifts
UCLAMP = 6.0     # clamp on u = 2.5*x  (=> (x+1)*2.5 <= 8.5, all bases 0 beyond)
EPS = 1e-5
NT = N // 128    # 8 token tiles


def _fold_w2(sw, ss):
    """(o,i,K=8) spline weights -> (o,i,12) relu^3-shift weights."""
    Wk = (sw * ss[..., None]).astype(np.float64)
    Cm = np.zeros((NS, 8))
    for k in range(8):
        for j in range(5):
            Cm[k + j, k] = ((-1) ** j) * math.comb(4, j) / 6.0
    return np.einsum('oik,sk->ois', Wk, Cm)


def _prep(inp):
    """Host-side parameter preparation (numpy, tiny)."""
    f = lambda a: np.ascontiguousarray(a, dtype=np.float32)
    g1, b1 = np.asarray(inp['g1'], np.float64), np.asarray(inp['b1'], np.float64)
    w_qkv = np.asarray(inp['w_qkv'], np.float64)
    sc = float(np.asarray(inp['sc']))

    # LN1 gamma/beta folded into qkv weights: row j of w_qkv acts on n1 = n0*g1+b1
    wq_g = w_qkv * g1[None, :]          # (144, 48)
    wq_b = w_qkv @ b1                   # (144,)

    # qk_lhsT (49, 4, 128): [Qg0, Qg1, Kg0, Kg1]; col r=32*i+d (d<6) = head 4g+i dim d
    qk = np.zeros((49, 4, 128))
    for gi, (base, g) in enumerate([(0, 0), (0, 1), (C, 0), (C, 1)]):
        for i in range(4):
            h = 4 * g + i
            for d in range(HD):
                j = base + h * HD + d
                qk[:C, gi, 32 * i + d] = wq_g[j]
                qk[C, gi, 32 * i + d] = wq_b[j]

    # v_rhs (49, 56): col h*7+d -> v head h dim d; col h*7+6 -> ones (selector row 48)
    vr = np.zeros((49, 56))
    for h in range(H):
        for d in range(HD):
            j = 2 * C + h * HD + d
            vr[:C, h * 7 + d] = wq_g[j]
            vr[C, h * 7 + d] = wq_b[j]
        vr[C, h * 7 + 6] = 1.0

    # proj_rhs (128, 48): lhsT rows 0..47 attn_sumT -> w_proj.T ; row 48 ones -> b_proj ;
    # rows 49..63 ones*0 ; rows 64..111 xT -> I ; rows 112..127 ones*0
    w_proj = np.asarray(inp['w_proj'], np.float64)
    pr = np.zeros((128, C))
    pr[:C] = w_proj.T
    pr[C] = np.asarray(inp['b_proj'], np.float64)
    pr[64:64 + C] = np.eye(C)

    # KAN layer 1 (in=48): selector (49, 576) col r=s*48+i ; weights (576, 24)
    def kan_prep(nin, nout, bw, sw, ss, g, b):
        # LN gamma/beta for the layer input are applied on-device (g2/b2); here identity.
        sel = np.zeros((nin + 1, NS * nin))
        W2 = _fold_w2(np.asarray(sw, np.float64), np.asarray(ss, np.float64))  # (o,i,12)
        wflat = np.zeros((NS * nin, nout))
        for s in range(NS):
            for i in range(nin):
                r = s * nin + i
                sel[i, r] = 1.0
                sel[nin, r] = 5.5 - s   # t = u + (5.5 - s), u = 2.5*x (clamped)
                wflat[r] = W2[:, i, s]
        return sel, wflat

    sel1, w1 = kan_prep(C, HID, inp['bw1'], inp['sw1'], inp['ss1'], None, None)
    sel2, w2 = kan_prep(HID, C, inp['bw2'], inp['sw2'], inp['ss2'], None, None)

    # conv lhsT
    c1w = np.asarray(inp['conv1_w'], np.float64)  # (128, 48, 3)
    c1 = np.zeros((49, 3, 128))
    for d in range(3):
        c1[:C, d] = c1w[:, :, d].T
    c1[C, 1] = np.asarray(inp['conv1_b'], np.float64)
    c2w = np.asarray(inp['conv3_w'], np.float64)  # (48, 128, 3)
    c2 = np.zeros((128, 3, C))
    for d in range(3):
        c2[:, d] = c2w[:, :, d].T

    m16 = np.kron(np.eye(8), np.ones((WIN, WIN)))
    ident = np.eye(128)

    def chunked(w, nout):
        """(rows, nout) -> (128, nchunk, nout) zero-padded chunk layout."""
        rows = w.shape[0]
        nchunk = (rows + 127) // 128
        out = np.zeros((128, nchunk, nout))
        for c in range(nchunk):
            r0, r1 = 128 * c, min(128 * (c + 1), rows)
            out[0:r1 - r0, c] = w[r0:r1]
        return out

    h = lambda a: np.ascontiguousarray(np.asarray(a, np.float32).astype(ml_dtypes.bfloat16))
    return {
        'qk_lhsT': h(qk), 'v_rhs': h(vr), 'proj_rhs': f(pr),
        'kan1_sel': f(sel1), 'kan1_w': f(chunked(w1, HID)), 'kan1_bw': f(np.asarray(inp['bw1']).T),
        'kan2_sel': f(sel2), 'kan2_w': f(chunked(w2, C)), 'kan2_bw': f(np.asarray(inp['bw2']).T),
        'conv1_lhsT': h(c1), 'conv2_lhsT': h(c2),
        'conv2_bc': f(np.asarray(inp['conv3_b']).reshape(C, 1)),
        'g2c': f(np.asarray(inp['g2']).reshape(C, 1)), 'b2c': f(np.asarray(inp['b2']).reshape(C, 1)),
        'g1sc': f(sc * np.asarray(inp['g1'])), 'b1sc': f(sc * np.asarray(inp['b1'])),
        'm16': h(m16), 'ident': f(ident),
    }


_PROG_CACHE = {}


def _build():
    nc = bacc.Bacc("TRN2", target_bir_lowering=False, debug=False)

    # ---- external I/O ----
    x_d = nc.dram_tensor("x", [N, C], F32, kind="ExternalInput")
    p_d = {}
    nch1 = (NS * C + 127) // 128
    nch2 = (NS * HID + 127) // 128
    shapes = {
        'qk_lhsT': [49, 4, 128], 'v_rhs': [49, 56], 'proj_rhs': [128, C],
        'kan1_sel': [C + 1, NS * C], 'kan1_w': [128, nch1, HID], 'kan1_bw': [C, HID],
        'kan2_sel': [HID + 1, NS * HID], 'kan2_w': [128, nch2, C], 'kan2_bw': [HID, C],
        'conv1_lhsT': [49, 3, 128], 'conv2_lhsT': [128, 3, C],
        'conv2_bc': [C, 1],
        'g2c': [C, 1], 'b2c': [C, 1], 'g1sc': [C], 'b1sc': [C],
        'm16': [128, 128], 'ident': [128, 128],
    }
    BF16_INPUTS = {'qk_lhsT', 'v_rhs', 'conv1_lhsT', 'conv2_lhsT', 'm16'}
    for k, s in shapes.items():
        dt = BF16 if k in BF16_INPUTS else F32
        p_d[k] = nc.dram_tensor(k, s, dt, kind="ExternalInput")
    attn_d = nc.dram_tensor("attn", [H, N, N], F32, kind="ExternalOutput")
    out_d = nc.dram_tensor("out", [N, C], F32, kind="ExternalOutput")

    r32 = lambda ap: ap.bitcast(F32R)

    with tile.TileContext(nc) as tc:
        with (
            tc.tile_pool(name="const", bufs=1) as cpool,
            tc.tile_pool(name="persist", bufs=1) as pp,
            tc.tile_pool(name="pt", bufs=6) as ptp,
            tc.tile_pool(name="po", bufs=4) as pop,
            tc.tile_pool(name="lm", bufs=2) as lmp,
            tc.tile_pool(name="r3", bufs=2) as r3p,
            tc.tile_pool(name="tiny", bufs=4) as tinyp,
            tc.tile_pool(name="psb", bufs=4, space="PSUM") as psb,   # (128,512) one-bank rotation
            tc.tile_pool(name="psa", bufs=1, space="PSUM") as psa,   # (128,1024) accumulators
            tc.tile_pool(name="pss", bufs=2, space="PSUM") as pss,   # (128,128) small
        ):
            # ---- load consts ----
            cs = {}
            for k, s in shapes.items():
                dt = BF16 if k in BF16_INPUTS else F32
                t = cpool.tile(s, dt, tag=k)
                nc.sync.dma_start(out=t[...], in_=p_d[k][...])
                cs[k] = t
            ident = cs['ident']

            eps_t = cpool.tile([128, 1], F32, tag="eps_t")
            nc.vector.memset(eps_t[:, :], EPS)

            # broadcast (C,) -> (128, C) tiles for final combine
            g1sc_b = cpool.tile([128, C], F32, tag="g1sc_b")
            b1sc_b = cpool.tile([128, C], F32, tag="b1sc_b")
            for name, dst in [('g1sc', g1sc_b), ('b1sc', b1sc_b)]:
                src = p_d[name][:]
                bc = bass.AP(tensor=src.tensor, offset=src.offset, ap=[[0, 128]] + list(src.ap))
                nc.gpsimd.dma_start(out=dst[...], in_=bc)

            # ---- persistent sbuf ----
            x_sb = pp.tile([128, NT, C], F32, tag="x_sb")
            nc.sync.dma_start(out=x_sb[...], in_=x_d[:].rearrange("(j p) c -> p j c", p=128))

            n10T = pp.tile([49, N], BF16, tag="n10T")      # row 48 = ones
            xT_pad = pp.tile([49, N + 2], BF16, tag="xT_pad")  # row 48 = ones (conv1 bias)
            proj_lhsT = pp.tile([128, N], F32, tag="proj_lhsT")
            Qp = [pp.tile([128, N], BF16, tag=f"Qp{g}", name=f"Qp{g}") for g in range(2)]
            Kp = [pp.tile([128, N], BF16, tag=f"Kp{g}", name=f"Kp{g}") for g in range(2)]
            V_sb = pp.tile([128, NT, 56], BF16, tag="V_sb")
            XgT = [pp.tile([128, N], F32, tag=f"XgT{g}", name=f"XgT{g}") for g in range(2)]
            XlT = [pp.tile([128, N], F32, tag=f"XlT{g}", name=f"XlT{g}") for g in range(2)]
            attn_sum = pp.tile([128, NT, C], F32, tag="attn_sum")
            x1_sb = pp.tile([128, NT, C], F32, tag="x1_sb")
            x2_sb = pp.tile([128, NT, C], F32, tag="x2_sb")
            out_sb = pp.tile([128, NT, C], F32, tag="out_sb")
            n2T = pp.tile([C, N], F32, tag="n2T")
            u1_aug = pp.tile([C + 1, N], F32, tag="u1_aug")    # row 48 = ones
            silu1 = pp.tile([C, N], F32, tag="silu1")
            kanhT = pp.tile([HID, N], F32, tag="kanhT")
            u2_aug = pp.tile([HID + 1, N], F32, tag="u2_aug")  # row 24 = ones
            silu2 = pp.tile([HID, N], F32, tag="silu2")
            kanoT = pp.tile([C, N], F32, tag="kanoT")
            h1_pad = pp.tile([128, N + 2], BF16, tag="h1_pad")
            h2T = pp.tile([C, N], F32, tag="h2T")

            # whole-tile memsets to 1.0; data rows are overwritten later, so the
            # "ones" rows survive (engine partition accesses must be 32-aligned,
            # so single-row memsets at partition 48/24 are not allowed).
            nc.vector.memset(n10T[:, :], 1.0)
            nc.vector.memset(u1_aug[:, :], 1.0)
            nc.vector.memset(u2_aug[:, :], 1.0)
            nc.vector.memset(proj_lhsT[:, :], 1.0)
            nc.vector.memset(xT_pad[:, :], 1.0)
            nc.vector.memset(xT_pad[:, 0:1], 0.0)
            nc.vector.memset(xT_pad[:, N + 1:N + 2], 0.0)
            nc.vector.memset(h1_pad[:, 0:1], 0.0)
            nc.vector.memset(h1_pad[:, N + 1:N + 2], 0.0)

            def ln_inv(mv):
                """per-partition 1/sqrt(var+eps) from bn_aggr output; returns (128,1) sbuf."""
                inv = tinyp.tile([128, 1], F32, tag="lninv")
                lnv = tinyp.tile([128, 1], F32, tag="lnvar")
                nc.scalar.activation(out=lnv[...], in_=mv[:, 1:2], func=AF.Ln,
                                     bias=eps_t[...])
                nc.scalar.activation(out=inv[...], in_=lnv[...], func=AF.Exp, scale=-0.5)
                return inv

            def ln_norm_tile(src_ap, dst_ap):
                """LayerNorm (g=identity) of a (128, C) tile: dst = (src-mu)*inv."""
                st = tinyp.tile([128, 6], F32, tag="bnst")
                mv = tinyp.tile([128, 2], F32, tag="bnmv")
                nc.vector.bn_stats(out=st[...], in_=src_ap)
                nc.vector.bn_aggr(out=mv[...], in_=st[...])
                inv = ln_inv(mv)
                nc.vector.tensor_scalar(out=dst_ap, in0=src_ap, scalar1=mv[:, 0:1],
                                        scalar2=inv[...], op0=ALU.subtract, op1=ALU.mult)
                return mv, inv

            # ---- stage 1+2: LN1, transposes of n1 and x ----
            for j in range(NT):
                n1t = tinyp.tile([128, C], F32, tag="n1t")
                ln_norm_tile(x_sb[:, j, :], n1t[...])
                tp = pss.tile([C, 128], F32, tag="sm")
                nc.tensor.transpose(tp[...], n1t[...], ident[...])
                nc.vector.tensor_copy(n10T[0:C, 128 * j:128 * (j + 1)], tp[...])
                tx = pss.tile([C, 128], F32, tag="sm")
                nc.tensor.transpose(tx[...], x_sb[:, j, :], ident[...])
                nc.vector.tensor_copy(xT_pad[0:C, 1 + 128 * j:1 + 128 * (j + 1)], tx[...])
                nc.vector.tensor_copy(proj_lhsT[64:64 + C, 128 * j:128 * (j + 1)], tx[...])

            # ---- stage 3: QKV + V ----
            for gi in range(4):
                dst = [Qp[0], Qp[1], Kp[0], Kp[1]][gi]
                for n in range(2):
                    qp = psb.tile([128, 512], F32, tag="s5", name="qp")
                    nc.tensor.matmul(qp[:, :],
                                     cs['qk_lhsT'][:, gi, :],
                                     n10T[:, 512 * n:512 * (n + 1)],
                                     start=True, stop=True)
                    nc.vector.tensor_copy(dst[:, 512 * n:512 * (n + 1)], qp[:, :])
            for j in range(NT):
                vp = pss.tile([128, 56], F32, tag="sm")
                nc.tensor.matmul(vp[...], n10T[:, 128 * j:128 * (j + 1)],
                                 cs['v_rhs'][...], start=True, stop=True)
                nc.vector.tensor_copy(V_sb[:, j, :], vp[...])

            # ---- conv path (depends only on xT_pad; PE fills early gaps) ----
            for n in range(2):
                h1p = psb.tile([128, 512], F32, tag="s5", name="h1p")
                for d in range(3):
                    nc.tensor.matmul(h1p[:, :],
                                     cs['conv1_lhsT'][:, d, :],
                                     xT_pad[:, 512 * n + d:512 * n + d + 512],
                                     start=(d == 0), stop=(d == 2))
                nc.vector.tensor_copy(h1_pad[:, 1 + 512 * n:1 + 512 * (n + 1)], h1p[:, :])
            for n in range(2):
                h2p = psb.tile([128, 512], F32, tag="s5", name="h2p")
                for d in range(3):
                    nc.tensor.matmul(h2p[0:C, :],
                                     cs['conv2_lhsT'][:, d, :],
                                     h1_pad[:, 512 * n + d:512 * n + d + 512],
                                     start=(d == 0), stop=(d == 2))
                nc.vector.tensor_scalar_add(out=h2T[:, 512 * n:512 * (n + 1)],
                                            in0=h2p[0:C, :],
                                            scalar1=cs['conv2_bc'][...])

            # ---- stage 4: attention per head-group g (heads 4g..4g+3) ----
            for g in range(2):
                # --- phase A: S^T -> exp -> P^T ; A@V (+ones col) ; local masked A@V ---
                xacc = psa.tile([128, N], F32, tag="acc")

                def emit_av(kt, pts):
                    for i in range(4):
                        pt = pts[i]
                        for n in range(2):
                            nc.tensor.matmul(
                                xacc[32 * i:32 * i + 7, 512 * n:512 * (n + 1)],
                                V_sb[:, kt, 7 * (4 * g + i):7 * (4 * g + i) + 7],
                                pt[:, 512 * n:512 * (n + 1)],
                                start=(kt == 0), stop=(kt == NT - 1),
                                tile_position=(0, 32 * i))
                        lm = lmp.tile([128, 128], BF16, tag="lm", name="lm")
                        nc.gpsimd.tensor_mul(lm[:, :], pt[:, 128 * kt:128 * (kt + 1)], cs['m16'][:, :])
                        xl = pss.tile([128, 128], F32, tag="sm", name="xl")
                        nc.tensor.matmul(xl[32 * i:32 * i + 7, :],
                                         V_sb[:, kt, 7 * (4 * g + i):7 * (4 * g + i) + 7],
                                         lm[:, :],
                                         start=True, stop=True, tile_position=(0, 32 * i))
                        nc.vector.tensor_copy(XlT[g][32 * i:32 * i + 7, 128 * kt:128 * (kt + 1)],
                                              xl[32 * i:32 * i + 7, :])

                # software pipeline: AV for tile kt-1 is emitted after S/exp for
                # tile kt so the PE never stalls waiting on ACT's exp.
                prev = None
                for kt in range(NT):
                    pts = []
                    for i in range(4):
                        pt = ptp.tile([128, N], BF16, tag="pt")
                        for n in range(2):
                            st = psb.tile([128, 512], F32, tag="s5", name="st")
                            nc.tensor.matmul(
                                st[:, :],
                                Kp[g][32 * i:32 * i + HD, 128 * kt:128 * (kt + 1)],
                                Qp[g][32 * i:32 * i + HD, 512 * n:512 * (n + 1)],
                                start=True, stop=True, tile_position=(32 * i, 0))
                            nc.scalar.activation(out=pt[:, 512 * n:512 * (n + 1)],
                                                 in_=st[:, :], func=AF.Exp, scale=SCALE)
                        pts.append(pt)
                    if prev is not None:
                        emit_av(prev[0], prev[1])
                    prev = (kt, pts)
                emit_av(prev[0], prev[1])
                nc.vector.tensor_copy(XgT[g][...], xacc[...])

                # --- phase C: normalize x, S row-major -> normalized P -> HBM ---
                for qt in range(NT):
                    tpg = pss.tile([128, 128], F32, tag="sm")
                    nc.tensor.transpose(tpg[...], XgT[g][:, 128 * qt:128 * (qt + 1)], ident[...])
                    tpl = pss.tile([128, 128], F32, tag="sm")
                    nc.tensor.transpose(tpl[...], XlT[g][:, 128 * qt:128 * (qt + 1)], ident[...])
                    recg = tinyp.tile([128, 4], F32, tag="recg")
                    recl = tinyp.tile([128, 4], F32, tag="recl")
                    nc.vector.reciprocal(out=recg[...], in_=tpg[:, 6:128:32])
                    nc.vector.reciprocal(out=recl[...], in_=tpl[:, 6:128:32])
                    lnrn = tinyp.tile([128, 4], F32, tag="lnrn")
                    nc.scalar.activation(out=lnrn[...], in_=recg[...], func=AF.Ln)

                    # x_global/x_local extraction: cols 32i+d (d<6) times 1/r
                    xg_ap = tpg[:, :].rearrange("p (i dd) -> p i dd", i=4)[:, :, 0:6]
                    xl_ap = tpl[:, :].rearrange("p (i dd) -> p i dd", i=4)[:, :, 0:6]
                    _rg = recg[...]
                    _rl = recl[...]
                    rgb = bass.AP(tensor=_rg.tensor, offset=_rg.offset,
                                  ap=[list(_rg.ap[0]), [1, 4], [0, 6]])
                    rlb = bass.AP(tensor=_rl.tensor, offset=_rl.offset,
                                  ap=[list(_rl.ap[0]), [1, 4], [0, 6]])
                    xgt = tinyp.tile([128, 24], F32, tag="xgt")
                    xlt = tinyp.tile([128, 24], F32, tag="xlt")
                    nc.vector.tensor_tensor(out=xgt[...], in0=xg_ap, in1=rgb, op=ALU.mult)
                    nc.vector.tensor_tensor(out=xlt[...], in0=xl_ap, in1=rlb, op=ALU.mult)
                    nc.vector.tensor_add(attn_sum[:, qt, 24 * g:24 * (g + 1)], xgt[...], xlt[...])

                    for i in range(4):
                        po = pop.tile([128, N], F32, tag="po")
                        for n in range(2):
                            srow = psb.tile([128, 512], F32, tag="s5", name="srow")
                            nc.tensor.matmul(
                                srow[:, :],
                                Qp[g][32 * i:32 * i + HD, 128 * qt:128 * (qt + 1)],
                                Kp[g][32 * i:32 * i + HD, 512 * n:512 * (n + 1)],
                                start=True, stop=True, tile_position=(32 * i, 0))
                            nc.scalar.activation(out=po[:, 512 * n:512 * (n + 1)],
                                                 in_=srow[:, :], func=AF.Exp,
                                                 scale=SCALE, bias=lnrn[:, i:i + 1])
                        nc.sync.dma_start(
                            out=attn_d[4 * g + i, 128 * qt:128 * (qt + 1), :], in_=po[...])

            # ---- stage 5: attn transpose + proj + x1 ----
            for qt in range(NT):
                ta = pss.tile([C, 128], F32, tag="sm")
                nc.tensor.transpose(ta[...], attn_sum[:, qt, :], ident[...])
                nc.vector.tensor_copy(proj_lhsT[0:C, 128 * qt:128 * (qt + 1)], ta[...])
            for qt in range(NT):
                x1p = pss.tile([128, C], F32, tag="sm")
                nc.tensor.matmul(x1p[...], proj_lhsT[:, 128 * qt:128 * (qt + 1)],
                                 cs['proj_rhs'][...], start=True, stop=True)
                nc.vector.tensor_copy(x1_sb[:, qt, :], x1p[...])

            # ---- stage 6: LN2 -> n2T (with g2/b2 in transposed orientation) ----
            for qt in range(NT):
                n2t = tinyp.tile([128, C], F32, tag="n2t")
                ln_norm_tile(x1_sb[:, qt, :], n2t[...])
                tp2 = pss.tile([C, 128], F32, tag="sm")
                nc.tensor.transpose(tp2[...], n2t[...], ident[...])
                nc.vector.tensor_copy(n2T[:, 128 * qt:128 * (qt + 1)], tp2[...])
            nc.vector.tensor_scalar(out=n2T[...], in0=n2T[...], scalar1=cs['g2c'][...],
                                    scalar2=cs['b2c'][...], op0=ALU.mult, op1=ALU.add)

            def silu_T(z, dst, nrow):
                """dst = z * sigmoid(z), rows (nrow, N), via exp/ln only."""
                e1 = tinyp.tile([nrow, N], F32, tag=f"se{nrow}", bufs=1, name="se")
                nc.scalar.activation(out=e1[...], in_=z, func=AF.Exp, scale=-1.0)
                sp = tinyp.tile([nrow, N], F32, tag=f"sp{nrow}", bufs=1, name="sp")
                nc.scalar.activation(out=sp[...], in_=e1[...], func=AF.Ln, bias=1.0)
                nc.scalar.activation(out=e1[...], in_=sp[...], func=AF.Exp, scale=-1.0)
                nc.vector.tensor_tensor(out=dst, in0=z, in1=e1[...], op=ALU.mult)

            def kan_layer(inT, nin, nout, sel, wmain, bw, siluT_dst, u_aug, out_psum_tag):
                """inT (nin, N) -> returns psum tile (nout, N) = base+spline."""
                silu_T(inT, siluT_dst[...], nin)
                nc.vector.tensor_scalar(out=u_aug[0:nin, :], in0=inT, scalar1=2.5,
                                        scalar2=UCLAMP, op0=ALU.mult, op1=ALU.min)
                nrows = NS * nin
                nchunk = (nrows + 127) // 128
                op = psa.tile([nout, N], F32, tag="acc")
                # base (silu) contribution first
                for n in range(2):
                    nc.tensor.matmul(op[:, 512 * n:512 * (n + 1)], bw[...],
                                     siluT_dst[:, 512 * n:512 * (n + 1)],
                                     start=True, stop=False,
                                     skip_group_check=True)
                for c in range(nchunk):
                    r0, r1 = 128 * c, min(128 * (c + 1), nrows)
                    rl = r3p.tile([128, N], F32, tag="r3")
                    for n in range(2):
                        tps = psb.tile([128, 512], F32, tag="s5", name="tps")
                        nc.tensor.matmul(tps[0:r1 - r0, :],
                                         sel[:, r0:r1],
                                         u_aug[:, 512 * n:512 * (n + 1)],
                                         start=True, stop=True)
                        nc.vector.tensor_scalar_max(out=rl[0:r1 - r0, 512 * n:512 * (n + 1)],
                                                    in0=tps[0:r1 - r0, :], scalar1=0.0)
                    sq = r3p.tile([128, N], F32, tag="r3sq")
                    nc.gpsimd.tensor_mul(sq[0:r1 - r0, :], rl[0:r1 - r0, :], rl[0:r1 - r0, :])
                    nc.gpsimd.tensor_mul(rl[0:r1 - r0, :], sq[0:r1 - r0, :], rl[0:r1 - r0, :])
                    for n in range(2):
                        nc.tensor.matmul(op[:, 512 * n:512 * (n + 1)],
                                         wmain[0:r1 - r0, c, :],
                                         rl[0:r1 - r0, 512 * n:512 * (n + 1)],
                                         start=False, stop=(c == nchunk - 1 and n == 1),
                                         skip_group_check=True)
                return op

            khp = kan_layer(n2T[...], C, HID, cs['kan1_sel'], cs['kan1_w'], cs['kan1_bw'],
                            silu1, u1_aug, "k1")
            nc.vector.tensor_copy(kanhT[...], khp[0:HID, :])
            kop = kan_layer(kanhT[...], HID, C, cs['kan2_sel'], cs['kan2_w'], cs['kan2_bw'],
                            silu2, u2_aug, "k2")
            nc.vector.tensor_copy(kanoT[...], kop[0:C, :])

            # ---- stage 9: x2 = x1 + kan_o ----
            for qt in range(NT):
                tk = pss.tile([128, C], F32, tag="sm")
                nc.tensor.transpose(tk[...], kanoT[:, 128 * qt:128 * (qt + 1)],
                                    ident[0:C, 0:C])
                nc.vector.tensor_add(x2_sb[:, qt, :], x1_sb[:, qt, :], tk[...])

            # ---- stage 11: conv LN + final combine ----
            for qt in range(NT):
                th = pss.tile([128, C], F32, tag="sm")
                nc.tensor.transpose(th[...], h2T[:, 128 * qt:128 * (qt + 1)],
                                    ident[0:C, 0:C])
                hn = tinyp.tile([128, C], F32, tag="hn")
                ln_norm_tile(th[...], hn[...])
                hterm = tinyp.tile([128, C], F32, tag="hterm")
                nc.vector.tensor_mul(hterm[...], hn[...], g1sc_b[...])
                nc.vector.tensor_add(hterm[...], hterm[...], b1sc_b[...])
                nc.vector.tensor_add(out_sb[:, qt, :], x2_sb[:, qt, :], hterm[...])
            nc.sync.dma_start(out=out_d[:].rearrange("(j p) c -> p j c", p=128), in_=out_sb[...])

    nc.compile()
    return nc


def kernel(**inputs):
    if 'prog' not in _PROG_CACHE:
        _PROG_CACHE['prog'] = _build()
    nc = _PROG_CACHE['prog']
    prep = _prep(inputs)
    x = np.ascontiguousarray(np.asarray(inputs['x'], np.float32))
    in_maps = []
    for b in range(B):
        m = dict(prep)
        m['x'] = np.ascontiguousarray(x[b])
        in_maps.append(m)
    res = run_bass_kernel_spmd(nc, in_maps, list(range(B)))
    out = np.stack([res.results[b]['out'] for b in range(B)])
    attn = np.stack([res.results[b]['attn'] for b in range(B)])
    return out, attn


# revision 32
# speedup vs baseline: 1.0334x; 1.0334x over previous
"""Trainium2 Bass kernel for nn_Block_37383395345129 (sparse_attention).

Data-parallel over batch B=8: one batch element per NeuronCore. Params
replicated. Per core: LN1 -> QKV -> global+local window attention
(returns softmax probs as output #2) -> proj residual -> LN2 -> KAN MLP
(B-spline basis via shifted relu^3 identity folded into weights) ->
conv residual path -> combine.

Key device-side tricks:
  - S and S^T both computed on PE (bf16 operands, fp32 PSUM accumulate,
    1 cyc/col); softmax without max-subtraction (|S*scale| < 1.5); row
    sums r obtained free via a ones-column in the A@V matmul; attention
    probs written in one ACT pass as exp(scale*S - ln r) (bias =
    per-partition -ln r).
  - A@V computed transposed (X^T = V^T P^T) with 4 heads col-packed in
    one PSUM tile via tile_position; local window attention via a
    block-diagonal mask on the diagonal P^T blocks (gpsimd mul).
  - KAN spline: bases_k(x) = B3(2.5x+5.5-k'), B3 via 4th difference of
    relu^3 -> 12 shifted relu^3 channels, basis->weight fold done on
    host; cancellation-critical matmuls in full fp32.
  - Single ACT table set (exp/ln): silu = z*exp(-ln(1+exp(-z))),
    1/sigma = exp(-0.5*ln(var+eps)).
"""
import math
import ml_dtypes
import numpy as np

import concourse.bass as bass
import concourse.mybir as mybir
import concourse.tile as tile
from concourse import bacc
from concourse.bass_utils import run_bass_kernel_spmd

F32 = mybir.dt.float32
BF16 = mybir.dt.bfloat16
F32R = mybir.dt.float32r
AF = mybir.ActivationFunctionType
ALU = mybir.AluOpType

B, N, C = 8, 1024, 48
H, HD = 8, 6
SCALE = HD ** -0.5
WIN = 16
HID = 24
NS = 12          # number of relu^3 shifts
UCLAMP = 6.0     # clamp on u = 2.5*x  (=> (x+1)*2.5 <= 8.5, all bases 0 beyond)
EPS = 1e-5
NT = N // 128    # 8 token tiles


def _fold_w2(sw, ss):
    """(o,i,K=8) spline weights -> (o,i,12) relu^3-shift weights."""
    Wk = (sw * ss[..., None]).astype(np.float64)
    Cm = np.zeros((NS, 8))
    for k in range(8):
        for j in range(5):
            Cm[k + j, k] = ((-1) ** j) * math.comb(4, j) / 6.0
    return np.einsum('oik,sk->ois', Wk, Cm)


def _prep(inp):
    """Host-side parameter preparation (numpy, tiny)."""
    f = lambda a: np.ascontiguousarray(a, dtype=np.float32)
    g1, b1 = np.asarray(inp['g1'], np.float64), np.asarray(inp['b1'], np.float64)
    w_qkv = np.asarray(inp['w_qkv'], np.float64)
    sc = float(np.asarray(inp['sc']))

    # LN1 gamma/beta folded into qkv weights: row j of w_qkv acts on n1 = n0*g1+b1
    wq_g = w_qkv * g1[None, :]          # (144, 48)
    wq_b = w_qkv @ b1                   # (144,)

    # qk_lhsT (49, 4, 128): [Qg0, Qg1, Kg0, Kg1]; col r=32*i+d (d<6) = head 4g+i dim d
    qk = np.zeros((49, 4, 128))
    for gi, (base, g) in enumerate([(0, 0), (0, 1), (C, 0), (C, 1)]):
        for i in range(4):
            h = 4 * g + i
            for d in range(HD):
                j = base + h * HD + d
                qk[:C, gi, 32 * i + d] = wq_g[j]
                qk[C, gi, 32 * i + d] = wq_b[j]

    # v_rhs (49, 56): col h*7+d -> v head h dim d; col h*7+6 -> ones (selector row 48)
    vr = np.zeros((49, 56))
    for h in range(H):
        for d in range(HD):
            j = 2 * C + h * HD + d
            vr[:C, h * 7 + d] = wq_g[j]
            vr[C, h * 7 + d] = wq_b[j]
        vr[C, h * 7 + 6] = 1.0

    # proj_rhs (128, 48): lhsT rows 0..47 attn_sumT -> w_proj.T ; row 48 ones -> b_proj ;
    # rows 49..63 ones*0 ; rows 64..111 xT -> I ; rows 112..127 ones*0
    w_proj = np.asarray(inp['w_proj'], np.float64)
    pr = np.zeros((128, C))
    pr[:C] = w_proj.T
    pr[C] = np.asarray(inp['b_proj'], np.float64)
    pr[64:64 + C] = np.eye(C)

    # KAN layer 1 (in=48): selector (49, 576) col r=s*48+i ; weights (576, 24)
    def kan_prep(nin, nout, bw, sw, ss, g, b):
        # LN gamma/beta for the layer input are applied on-device (g2/b2); here identity.
        sel = np.zeros((nin + 1, NS * nin))
        W2 = _fold_w2(np.asarray(sw, np.float64), np.asarray(ss, np.float64))  # (o,i,12)
        wflat = np.zeros((NS * nin, nout))
        for s in range(NS):
            for i in range(nin):
                r = s * nin + i
                sel[i, r] = 1.0
                sel[nin, r] = 5.5 - s   # t = u + (5.5 - s), u = 2.5*x (clamped)
                wflat[r] = W2[:, i, s]
        return sel, wflat

    sel1, w1 = kan_prep(C, HID, inp['bw1'], inp['sw1'], inp['ss1'], None, None)
    sel2, w2 = kan_prep(HID, C, inp['bw2'], inp['sw2'], inp['ss2'], None, None)

    # conv lhsT
    c1w = np.asarray(inp['conv1_w'], np.float64)  # (128, 48, 3)
    c1 = np.zeros((49, 3, 128))
    for d in range(3):
        c1[:C, d] = c1w[:, :, d].T
    c1[C, 1] = np.asarray(inp['conv1_b'], np.float64)
    c2w = np.asarray(inp['conv3_w'], np.float64)  # (48, 128, 3)
    c2 = np.zeros((128, 3, C))
    for d in range(3):
        c2[:, d] = c2w[:, :, d].T

    m16 = np.kron(np.eye(8), np.ones((WIN, WIN)))
    ident = np.eye(128)

    def chunked(w, nout):
        """(rows, nout) -> (128, nchunk, nout) zero-padded chunk layout."""
        rows = w.shape[0]
        nchunk = (rows + 127) // 128
        out = np.zeros((128, nchunk, nout))
        for c in range(nchunk):
            r0, r1 = 128 * c, min(128 * (c + 1), rows)
            out[0:r1 - r0, c] = w[r0:r1]
        return out

    h = lambda a: np.ascontiguousarray(np.asarray(a, np.float32).astype(ml_dtypes.bfloat16))
    return {
        'qk_lhsT': h(qk), 'v_rhs': h(vr), 'proj_rhs': f(pr),
        'kan1_sel': f(sel1), 'kan1_w': f(chunked(w1, HID)), 'kan1_bw': f(np.asarray(inp['bw1']).T),
        'kan2_sel': f(sel2), 'kan2_w': f(chunked(w2, C)), 'kan2_bw': f(np.asarray(inp['bw2']).T),
        'conv1_lhsT': h(c1), 'conv2_lhsT': h(c2),
        'conv2_bc': f(np.asarray(inp['conv3_b']).reshape(C, 1)),
        'g2c': f(np.asarray(inp['g2']).reshape(C, 1)), 'b2c': f(np.asarray(inp['b2']).reshape(C, 1)),
        'g1sc': f(sc * np.asarray(inp['g1'])), 'b1sc': f(sc * np.asarray(inp['b1'])),
        'm16': h(m16), 'ident': f(ident),
    }


_PROG_CACHE = {}


def _build():
    nc = bacc.Bacc("TRN2", target_bir_lowering=False, debug=False)

    # ---- external I/O ----
    x_d = nc.dram_tensor("x", [N, C], F32, kind="ExternalInput")
    p_d = {}
    nch1 = (NS * C + 127) // 128
    nch2 = (NS * HID + 127) // 128
    shapes = {
        'qk_lhsT': [49, 4, 128], 'v_rhs': [49, 56], 'proj_rhs': [128, C],
        'kan1_sel': [C + 1, NS * C], 'kan1_w': [128, nch1, HID], 'kan1_bw': [C, HID],
        'kan2_sel': [HID + 1, NS * HID], 'kan2_w': [128, nch2, C], 'kan2_bw': [HID, C],
        'conv1_lhsT': [49, 3, 128], 'conv2_lhsT': [128, 3, C],
        'conv2_bc': [C, 1],
        'g2c': [C, 1], 'b2c': [C, 1], 'g1sc': [C], 'b1sc': [C],
        'm16': [128, 128], 'ident': [128, 128],
    }
    BF16_INPUTS = {'qk_lhsT', 'v_rhs', 'conv1_lhsT', 'conv2_lhsT', 'm16'}
    for k, s in shapes.items():
        dt = BF16 if k in BF16_INPUTS else F32
        p_d[k] = nc.dram_tensor(k, s, dt, kind="ExternalInput")
    attn_d = nc.dram_tensor("attn", [H, N, N], F32, kind="ExternalOutput")
    out_d = nc.dram_tensor("out", [N, C], F32, kind="ExternalOutput")

    r32 = lambda ap: ap.bitcast(F32R)

    with tile.TileContext(nc) as tc:
        with (
            tc.tile_pool(name="const", bufs=1) as cpool,
            tc.tile_pool(name="persist", bufs=1) as pp,
            tc.tile_pool(name="pt", bufs=9) as ptp,
            tc.tile_pool(name="po", bufs=6) as pop,
            tc.tile_pool(name="lm", bufs=3) as lmp,
            tc.tile_pool(name="r3", bufs=3) as r3p,
            tc.tile_pool(name="tiny", bufs=4) as tinyp,
            tc.tile_pool(name="psb", bufs=4, space="PSUM") as psb,   # (128,512) one-bank rotation
            tc.tile_pool(name="psa", bufs=1, space="PSUM") as psa,   # (128,1024) accumulators
            tc.tile_pool(name="pss", bufs=2, space="PSUM") as pss,   # (128,128) small
        ):
            # ---- load consts ----
            cs = {}
            for k, s in shapes.items():
                dt = BF16 if k in BF16_INPUTS else F32
                t = cpool.tile(s, dt, tag=k)
                nc.sync.dma_start(out=t[...], in_=p_d[k][...])
                cs[k] = t
            ident = cs['ident']

            eps_t = cpool.tile([128, 1], F32, tag="eps_t")
            nc.vector.memset(eps_t[:, :], EPS)

            # broadcast (C,) -> (128, C) tiles for final combine
            g1sc_b = cpool.tile([128, C], F32, tag="g1sc_b")
            b1sc_b = cpool.tile([128, C], F32, tag="b1sc_b")
            for name, dst in [('g1sc', g1sc_b), ('b1sc', b1sc_b)]:
                src = p_d[name][:]
                bc = bass.AP(tensor=src.tensor, offset=src.offset, ap=[[0, 128]] + list(src.ap))
                nc.gpsimd.dma_start(out=dst[...], in_=bc)

            # ---- persistent sbuf ----
            x_sb = pp.tile([128, NT, C], F32, tag="x_sb")
            nc.sync.dma_start(out=x_sb[...], in_=x_d[:].rearrange("(j p) c -> p j c", p=128))

            n10T = pp.tile([49, N], BF16, tag="n10T")      # row 48 = ones
            xT_pad = pp.tile([49, N + 2], BF16, tag="xT_pad")  # row 48 = ones (conv1 bias)
            proj_lhsT = pp.tile([128, N], F32, tag="proj_lhsT")
            Qp = [pp.tile([128, N], BF16, tag=f"Qp{g}", name=f"Qp{g}") for g in range(2)]
            Kp = [pp.tile([128, N], BF16, tag=f"Kp{g}", name=f"Kp{g}") for g in range(2)]
            V_sb = pp.tile([128, NT, 56], BF16, tag="V_sb")
            XgT = [pp.tile([128, N], F32, tag=f"XgT{g}", name=f"XgT{g}") for g in range(2)]
            XlT = [pp.tile([128, N], F32, tag=f"XlT{g}", name=f"XlT{g}") for g in range(2)]
            attn_sum = pp.tile([128, NT, C], F32, tag="attn_sum")
            x1_sb = pp.tile([128, NT, C], F32, tag="x1_sb")
            x2_sb = pp.tile([128, NT, C], F32, tag="x2_sb")
            out_sb = pp.tile([128, NT, C], F32, tag="out_sb")
            n2T = pp.tile([C, N], F32, tag="n2T")
            u1_aug = pp.tile([C + 1, N], F32, tag="u1_aug")    # row 48 = ones
            silu1 = pp.tile([C, N], F32, tag="silu1")
            kanhT = pp.tile([HID, N], F32, tag="kanhT")
            u2_aug = pp.tile([HID + 1, N], F32, tag="u2_aug")  # row 24 = ones
            silu2 = pp.tile([HID, N], F32, tag="silu2")
            kanoT = pp.tile([C, N], F32, tag="kanoT")
            h1_pad = pp.tile([128, N + 2], BF16, tag="h1_pad")
            h2T = pp.tile([C, N], F32, tag="h2T")

            # whole-tile memsets to 1.0; data rows are overwritten later, so the
            # "ones" rows survive (engine partition accesses must be 32-aligned,
            # so single-row memsets at partition 48/24 are not allowed).
            nc.vector.memset(n10T[:, :], 1.0)
            nc.vector.memset(u1_aug[:, :], 1.0)
            nc.vector.memset(u2_aug[:, :], 1.0)
            nc.vector.memset(proj_lhsT[:, :], 1.0)
            nc.vector.memset(xT_pad[:, :], 1.0)
            nc.vector.memset(xT_pad[:, 0:1], 0.0)
            nc.vector.memset(xT_pad[:, N + 1:N + 2], 0.0)
            nc.vector.memset(h1_pad[:, 0:1], 0.0)
            nc.vector.memset(h1_pad[:, N + 1:N + 2], 0.0)

            def ln_inv(mv):
                """per-partition 1/sqrt(var+eps) from bn_aggr output; returns (128,1) sbuf."""
                inv = tinyp.tile([128, 1], F32, tag="lninv")
                lnv = tinyp.tile([128, 1], F32, tag="lnvar")
                nc.scalar.activation(out=lnv[...], in_=mv[:, 1:2], func=AF.Ln,
                                     bias=eps_t[...])
                nc.scalar.activation(out=inv[...], in_=lnv[...], func=AF.Exp, scale=-0.5)
                return inv

            def ln_norm_tile(src_ap, dst_ap):
                """LayerNorm (g=identity) of a (128, C) tile: dst = (src-mu)*inv."""
                st = tinyp.tile([128, 6], F32, tag="bnst")
                mv = tinyp.tile([128, 2], F32, tag="bnmv")
                nc.vector.bn_stats(out=st[...], in_=src_ap)
                nc.vector.bn_aggr(out=mv[...], in_=st[...])
                inv = ln_inv(mv)
                nc.vector.tensor_scalar(out=dst_ap, in0=src_ap, scalar1=mv[:, 0:1],
                                        scalar2=inv[...], op0=ALU.subtract, op1=ALU.mult)
                return mv, inv

            # ---- stage 1+2: LN1, transposes of n1 and x ----
            for j in range(NT):
                n1t = tinyp.tile([128, C], F32, tag="n1t")
                ln_norm_tile(x_sb[:, j, :], n1t[...])
                tp = pss.tile([C, 128], F32, tag="sm")
                nc.tensor.transpose(tp[...], n1t[...], ident[...])
                nc.vector.tensor_copy(n10T[0:C, 128 * j:128 * (j + 1)], tp[...])
                tx = pss.tile([C, 128], F32, tag="sm")
                nc.tensor.transpose(tx[...], x_sb[:, j, :], ident[...])
                nc.vector.tensor_copy(xT_pad[0:C, 1 + 128 * j:1 + 128 * (j + 1)], tx[...])
                nc.vector.tensor_copy(proj_lhsT[64:64 + C, 128 * j:128 * (j + 1)], tx[...])

            # ---- stage 3: QKV + V ----
            for gi in range(4):
                dst = [Qp[0], Qp[1], Kp[0], Kp[1]][gi]
                for n in range(2):
                    qp = psb.tile([128, 512], F32, tag="s5", name="qp")
                    nc.tensor.matmul(qp[:, :],
                                     cs['qk_lhsT'][:, gi, :],
                                     n10T[:, 512 * n:512 * (n + 1)],
                                     start=True, stop=True)
                    nc.vector.tensor_copy(dst[:, 512 * n:512 * (n + 1)], qp[:, :])
            for j in range(NT):
                vp = pss.tile([128, 56], F32, tag="sm")
                nc.tensor.matmul(vp[...], n10T[:, 128 * j:128 * (j + 1)],
                                 cs['v_rhs'][...], start=True, stop=True)
                nc.vector.tensor_copy(V_sb[:, j, :], vp[...])

            # ---- conv path (depends only on xT_pad; PE fills early gaps) ----
            for n in range(2):
                h1p = psb.tile([128, 512], F32, tag="s5", name="h1p")
                for d in range(3):
                    nc.tensor.matmul(h1p[:, :],
                                     cs['conv1_lhsT'][:, d, :],
                                     xT_pad[:, 512 * n + d:512 * n + d + 512],
                                     start=(d == 0), stop=(d == 2))
                nc.vector.tensor_copy(h1_pad[:, 1 + 512 * n:1 + 512 * (n + 1)], h1p[:, :])
            for n in range(2):
                h2p = psb.tile([128, 512], F32, tag="s5", name="h2p")
                for d in range(3):
                    nc.tensor.matmul(h2p[0:C, :],
                                     cs['conv2_lhsT'][:, d, :],
                                     h1_pad[:, 512 * n + d:512 * n + d + 512],
                                     start=(d == 0), stop=(d == 2))
                nc.vector.tensor_scalar_add(out=h2T[:, 512 * n:512 * (n + 1)],
                                            in0=h2p[0:C, :],
                                            scalar1=cs['conv2_bc'][...])

            # ---- stage 4: attention per head-group g (heads 4g..4g+3) ----
            for g in range(2):
                # --- phase A: S^T -> exp -> P^T ; A@V (+ones col) ; local masked A@V ---
                xacc = psa.tile([128, N], F32, tag="acc")

                def emit_av(kt, pts):
                    for i in range(4):
                        pt = pts[i]
                        for n in range(2):
                            nc.tensor.matmul(
                                xacc[32 * i:32 * i + 7, 512 * n:512 * (n + 1)],
                                V_sb[:, kt, 7 * (4 * g + i):7 * (4 * g + i) + 7],
                                pt[:, 512 * n:512 * (n + 1)],
                                start=(kt == 0), stop=(kt == NT - 1),
                                tile_position=(0, 32 * i))
                        lm = lmp.tile([128, 128], BF16, tag="lm", name="lm")
                        nc.gpsimd.tensor_mul(lm[:, :], pt[:, 128 * kt:128 * (kt + 1)], cs['m16'][:, :])
                        xl = pss.tile([128, 128], F32, tag="sm", name="xl")
                        nc.tensor.matmul(xl[32 * i:32 * i + 7, :],
                                         V_sb[:, kt, 7 * (4 * g + i):7 * (4 * g + i) + 7],
                                         lm[:, :],
                                         start=True, stop=True, tile_position=(0, 32 * i))
                        nc.vector.tensor_copy(XlT[g][32 * i:32 * i + 7, 128 * kt:128 * (kt + 1)],
                                              xl[32 * i:32 * i + 7, :])

                # software pipeline: AV for tile kt-1 is emitted after S/exp for
                # tile kt so the PE never stalls waiting on ACT's exp.
                prev = None
                for kt in range(NT):
                    pts = []
                    for i in range(4):
                        pt = ptp.tile([128, N], BF16, tag="pt")
                        for n in range(2):
                            st = psb.tile([128, 512], F32, tag="s5", name="st")
                            nc.tensor.matmul(
                                st[:, :],
                                Kp[g][32 * i:32 * i + HD, 128 * kt:128 * (kt + 1)],
                                Qp[g][32 * i:32 * i + HD, 512 * n:512 * (n + 1)],
                                start=True, stop=True, tile_position=(32 * i, 0))
                            nc.scalar.activation(out=pt[:, 512 * n:512 * (n + 1)],
                                                 in_=st[:, :], func=AF.Exp, scale=SCALE)
                        pts.append(pt)
                    if prev is not None:
                        emit_av(prev[0], prev[1])
                    prev = (kt, pts)
                emit_av(prev[0], prev[1])
                nc.vector.tensor_copy(XgT[g][...], xacc[...])

                # --- phase C: normalize x, S row-major -> normalized P -> HBM ---
                for qt in range(NT):
                    tpg = pss.tile([128, 128], F32, tag="sm")
                    nc.tensor.transpose(tpg[...], XgT[g][:, 128 * qt:128 * (qt + 1)], ident[...])
                    tpl = pss.tile([128, 128], F32, tag="sm")
                    nc.tensor.transpose(tpl[...], XlT[g][:, 128 * qt:128 * (qt + 1)], ident[...])
                    recg = tinyp.tile([128, 4], F32, tag="recg")
                    recl = tinyp.tile([128, 4], F32, tag="recl")
                    nc.vector.reciprocal(out=recg[...], in_=tpg[:, 6:128:32])
                    nc.vector.reciprocal(out=recl[...], in_=tpl[:, 6:128:32])
                    lnrn = tinyp.tile([128, 4], F32, tag="lnrn")
                    nc.scalar.activation(out=lnrn[...], in_=recg[...], func=AF.Ln)

                    # x_global/x_local extraction: cols 32i+d (d<6) times 1/r
                    xg_ap = tpg[:, :].rearrange("p (i dd) -> p i dd", i=4)[:, :, 0:6]
                    xl_ap = tpl[:, :].rearrange("p (i dd) -> p i dd", i=4)[:, :, 0:6]
                    _rg = recg[...]
                    _rl = recl[...]
                    rgb = bass.AP(tensor=_rg.tensor, offset=_rg.offset,
                                  ap=[list(_rg.ap[0]), [1, 4], [0, 6]])
                    rlb = bass.AP(tensor=_rl.tensor, offset=_rl.offset,
                                  ap=[list(_rl.ap[0]), [1, 4], [0, 6]])
                    xgt = tinyp.tile([128, 24], F32, tag="xgt")
                    xlt = tinyp.tile([128, 24], F32, tag="xlt")
                    nc.vector.tensor_tensor(out=xgt[...], in0=xg_ap, in1=rgb, op=ALU.mult)
                    nc.vector.tensor_tensor(out=xlt[...], in0=xl_ap, in1=rlb, op=ALU.mult)
                    nc.vector.tensor_add(attn_sum[:, qt, 24 * g:24 * (g + 1)], xgt[...], xlt[...])

                    for i in range(4):
                        po = pop.tile([128, N], F32, tag="po")
                        for n in range(2):
                            srow = psb.tile([128, 512], F32, tag="s5", name="srow")
                            nc.tensor.matmul(
                                srow[:, :],
                                Qp[g][32 * i:32 * i + HD, 128 * qt:128 * (qt + 1)],
                                Kp[g][32 * i:32 * i + HD, 512 * n:512 * (n + 1)],
                                start=True, stop=True, tile_position=(32 * i, 0))
                            nc.scalar.activation(out=po[:, 512 * n:512 * (n + 1)],
                                                 in_=srow[:, :], func=AF.Exp,
                                                 scale=SCALE, bias=lnrn[:, i:i + 1])
                        nc.sync.dma_start(
                            out=attn_d[4 * g + i, 128 * qt:128 * (qt + 1), :], in_=po[...])

            # ---- stage 5: attn transpose + proj + x1 ----
            for qt in range(NT):
                ta = pss.tile([C, 128], F32, tag="sm")
                nc.tensor.transpose(ta[...], attn_sum[:, qt, :], ident[...])
                nc.vector.tensor_copy(proj_lhsT[0:C, 128 * qt:128 * (qt + 1)], ta[...])
            for qt in range(NT):
                x1p = pss.tile([128, C], F32, tag="sm")
                nc.tensor.matmul(x1p[...], proj_lhsT[:, 128 * qt:128 * (qt + 1)],
                                 cs['proj_rhs'][...], start=True, stop=True)
                nc.vector.tensor_copy(x1_sb[:, qt, :], x1p[...])

            # ---- stage 6: LN2 -> n2T (with g2/b2 in transposed orientation) ----
            for qt in range(NT):
                n2t = tinyp.tile([128, C], F32, tag="n2t")
                ln_norm_tile(x1_sb[:, qt, :], n2t[...])
                tp2 = pss.tile([C, 128], F32, tag="sm")
                nc.tensor.transpose(tp2[...], n2t[...], ident[...])
                nc.vector.tensor_copy(n2T[:, 128 * qt:128 * (qt + 1)], tp2[...])
            nc.vector.tensor_scalar(out=n2T[...], in0=n2T[...], scalar1=cs['g2c'][...],
                                    scalar2=cs['b2c'][...], op0=ALU.mult, op1=ALU.add)

            def silu_T(z, dst, nrow):
                """dst = z * sigmoid(z), rows (nrow, N), via exp/ln only."""
                e1 = tinyp.tile([nrow, N], F32, tag=f"se{nrow}", bufs=1, name="se")
                nc.scalar.activation(out=e1[...], in_=z, func=AF.Exp, scale=-1.0)
                sp = tinyp.tile([nrow, N], F32, tag=f"sp{nrow}", bufs=1, name="sp")
                nc.scalar.activation(out=sp[...], in_=e1[...], func=AF.Ln, bias=1.0)
                nc.scalar.activation(out=e1[...], in_=sp[...], func=AF.Exp, scale=-1.0)
                nc.vector.tensor_tensor(out=dst, in0=z, in1=e1[...], op=ALU.mult)

            def kan_layer(inT, nin, nout, sel, wmain, bw, siluT_dst, u_aug, out_psum_tag):
                """inT (nin, N) -> returns psum tile (nout, N) = base+spline."""
                silu_T(inT, siluT_dst[...], nin)
                nc.vector.tensor_scalar(out=u_aug[0:nin, :], in0=inT, scalar1=2.5,
                                        scalar2=UCLAMP, op0=ALU.mult, op1=ALU.min)
                nrows = NS * nin
                nchunk = (nrows + 127) // 128
                op = psa.tile([nout, N], F32, tag="acc")
                # base (silu) contribution first
                for n in range(2):
                    nc.tensor.matmul(op[:, 512 * n:512 * (n + 1)], bw[...],
                                     siluT_dst[:, 512 * n:512 * (n + 1)],
                                     start=True, stop=False,
                                     skip_group_check=True)
                for c in range(nchunk):
                    r0, r1 = 128 * c, min(128 * (c + 1), nrows)
                    rl = r3p.tile([128, N], F32, tag="r3")
                    for n in range(2):
                        tps = psb.tile([128, 512], F32, tag="s5", name="tps")
                        nc.tensor.matmul(tps[0:r1 - r0, :],
                                         sel[:, r0:r1],
                                         u_aug[:, 512 * n:512 * (n + 1)],
                                         start=True, stop=True)
                        nc.vector.tensor_scalar_max(out=rl[0:r1 - r0, 512 * n:512 * (n + 1)],
                                                    in0=tps[0:r1 - r0, :], scalar1=0.0)
                    sq = r3p.tile([128, N], F32, tag="r3sq")
                    nc.gpsimd.tensor_mul(sq[0:r1 - r0, :], rl[0:r1 - r0, :], rl[0:r1 - r0, :])
                    nc.gpsimd.tensor_mul(rl[0:r1 - r0, :], sq[0:r1 - r0, :], rl[0:r1 - r0, :])
                    for n in range(2):
                        nc.tensor.matmul(op[:, 512 * n:512 * (n + 1)],
                                         wmain[0:r1 - r0, c, :],
                                         rl[0:r1 - r0, 512 * n:512 * (n + 1)],
                                         start=False, stop=(c == nchunk - 1 and n == 1),
                                         skip_group_check=True)
                return op

            khp = kan_layer(n2T[...], C, HID, cs['kan1_sel'], cs['kan1_w'], cs['kan1_bw'],
                            silu1, u1_aug, "k1")
            nc.vector.tensor_copy(kanhT[...], khp[0:HID, :])
            kop = kan_layer(kanhT[...], HID, C, cs['kan2_sel'], cs['kan2_w'], cs['kan2_bw'],
                            silu2, u2_aug, "k2")
            nc.vector.tensor_copy(kanoT[...], kop[0:C, :])

            # ---- stage 9: x2 = x1 + kan_o ----
            for qt in range(NT):
                tk = pss.tile([128, C], F32, tag="sm")
                nc.tensor.transpose(tk[...], kanoT[:, 128 * qt:128 * (qt + 1)],
                                    ident[0:C, 0:C])
                nc.vector.tensor_add(x2_sb[:, qt, :], x1_sb[:, qt, :], tk[...])

            # ---- stage 11: conv LN + final combine ----
            for qt in range(NT):
                th = pss.tile([128, C], F32, tag="sm")
                nc.tensor.transpose(th[...], h2T[:, 128 * qt:128 * (qt + 1)],
                                    ident[0:C, 0:C])
                hn = tinyp.tile([128, C], F32, tag="hn")
                ln_norm_tile(th[...], hn[...])
                hterm = tinyp.tile([128, C], F32, tag="hterm")
                nc.vector.tensor_mul(hterm[...], hn[...], g1sc_b[...])
                nc.vector.tensor_add(hterm[...], hterm[...], b1sc_b[...])
                nc.vector.tensor_add(out_sb[:, qt, :], x2_sb[:, qt, :], hterm[...])
            nc.sync.dma_start(out=out_d[:].rearrange("(j p) c -> p j c", p=128), in_=out_sb[...])

    nc.compile()
    return nc


def kernel(**inputs):
    if 'prog' not in _PROG_CACHE:
        _PROG_CACHE['prog'] = _build()
    nc = _PROG_CACHE['prog']
    prep = _prep(inputs)
    x = np.ascontiguousarray(np.asarray(inputs['x'], np.float32))
    in_maps = []
    for b in range(B):
        m = dict(prep)
        m['x'] = np.ascontiguousarray(x[b])
        in_maps.append(m)
    res = run_bass_kernel_spmd(nc, in_maps, list(range(B)))
    out = np.stack([res.results[b]['out'] for b in range(B)])
    attn = np.stack([res.results[b]['attn'] for b in range(B)])
    return out, attn
